# revision 6
# baseline (speedup 1.0000x reference)
"""Trainium2 kernel for the DDC sequential-scan model (8 NeuronCores).

x_{t+1} = (T_base + sum_a act[t,a] * A_mats[a]) @ x_t + b ;  reward[t] = r . x_{t+1}

Strategy: row-shard all 5 matrices (output dim) across the 8 cores
(512 rows/core), per the tensor-parallel sharding hint. Each of the 50
strictly-sequential steps computes the local 512-row shard of the new
interface with f16 matvecs (f32 accumulate on the PE array), applies the
action-conditioned combine + bias in f32, all-gathers the 4096-vector
(2 KB/rank, intra-chip) to rebuild the carried interface on every core,
and computes the reward redundantly per core (no extra collective).
Weights are stored f16: half the HBM traffic of f32, and the 50-step
chain keeps rel-err ~1e-3 vs the f32 oracle (validated numerically).
The step loop is fully unrolled: jax.lax.scan on this backend
miscompiles per-iteration reward extraction (steps 32/49 return zero).

Note: the bass/walrus NEFF path (`bass_utils.run_bass_kernel_spmd`)
cannot be used for the cross-core exchange in this axon-tunneled
environment: NEFFs containing ncfw collectives fail at LoadExecutable,
and remote_dma SWDGE frames fault at execution (both verified against a
working XLA psum on the same 8 cores). The kernel therefore drives the
same 8 NeuronCores through the neuron PJRT backend, the only
collective-capable path available here.
"""
import numpy as np

N = 4096
L = 50
A_NUM = 4
NCORES = 8
SHARD = N // NCORES  # 512

_cache = {}


def _get_fn():
    if "fn" in _cache:
        return _cache["fn"]
    import jax
    import jax.numpy as jnp
    from jax.sharding import Mesh, PartitionSpec as P
    from jax.experimental.shard_map import shard_map

    devs = jax.devices()[:NCORES]
    assert len(devs) >= NCORES, f"need {NCORES} devices, got {len(devs)}"
    mesh = Mesh(np.array(devs[:NCORES]), ("c",))

    def percore(Tl, Al, bsh, trajv, rv):
        # Tl (512, 4096) f16, Al (4, 512, 4096) f16: this core's row shards
        # bsh (512,) f32 local bias shard; trajv (50,4) f32; rv (4096,) f32
        Aflat = Al.reshape(A_NUM * SHARD, N)
        x = jnp.zeros((N,), jnp.float32)
        xs = []
        for t in range(L):
            xh = x.astype(jnp.float16)
            yT = jnp.matmul(Tl, xh).astype(jnp.float32)             # (512,)
            yA = jnp.matmul(Aflat, xh).astype(jnp.float32)          # (2048,)
            local = yT + jnp.tensordot(trajv[t], yA.reshape(A_NUM, SHARD), axes=1) + bsh
            x = jax.lax.all_gather(local, "c", tiled=True)          # (4096,)
            xs.append(x)
        return jnp.stack(xs) @ rv  # (50,)

    fn = jax.jit(shard_map(
        percore, mesh=mesh,
        in_specs=(P("c"), P(None, "c"), P("c"), P(), P()),
        out_specs=P(),
        check_rep=False,
    ))
    _cache["fn"] = fn
    _cache["mesh"] = mesh
    return fn


def kernel(init_states, trajectories, T_base, A_mats, b, r):
    fn = _get_fn()
    Th = np.asarray(T_base).astype(np.float16)           # (4096, 4096)
    Ah = np.asarray(A_mats).astype(np.float16)           # (4, 4096, 4096)
    out = fn(
        Th, Ah,
        np.asarray(b, np.float32),
        np.asarray(trajectories, np.float32),
        np.asarray(r, np.float32),
    )
    return np.asarray(out, dtype=np.float32)
